# revision 14
# baseline (speedup 1.0000x reference)
"""Trainium2 Bass kernel for nn_PositionalEncoding (gnn_message_passing).

Self-contained: takes FULL inputs, shards across 8 NeuronCores internally,
runs one SPMD Bass program, reassembles the full output on the host.

Math (per reference):
  deg  = relu(deg_emb[tree_degree] @ W1 + b1)
  x    = (x_clique + deg) @ Wm + mb
  tpe  = nan0(tree_lpe) @ tlw + tlb
  pe   = nan0(graph_lpe) @ lpw + lpb
  pec  = segment_mean(pe[row], col)        (0 where count==0)
  out  = x + concat([pec, tpe], -1)

v3 device strategy (everything in [feat, clique] space):
  - cliques sorted by edge-count k into uniform classes (host index prep)
  - per group of 4 tiles (512 cliques): ONE PSUM bank accumulates
      x@Wm (bf16, start=True covers whole bank -> no memset)
    + T2[deg] via one-hot matmul: host formats deg as an fp8 one-hot
      stream [100, NP]; T2 = relu(deg_emb@W1+b1)@Wm computed on device
    + tpe into rows 64:128 (bf16)
    + pec into rows 0:64: host pre-gathers graph_lpe edge rows (fp8 e3m4)
      into exact-height chunk streams [h*32 partitions, cliques] for
      h = 4,3,2,1 slots; a stacked weight [lpw x h]*(1/k) contracts
      slot-sum AND projection in one matmul per chunk (no reduces,
      no transposes, no slot zero-padding)
  - drain PSUM -> bf16 SBUF with fused bias add on ScalarE (DVE idle)
  - DMA in 4096-col (~1MB) slabs, double buffered
"""

import math

import numpy as np
import ml_dtypes

BF16 = ml_dtypes.bfloat16
FP8 = ml_dtypes.float8_e3m4

N_CORES = 8
HID = 128
PE = 32
P = 128        # partitions / clique-tile size
GROUP = 4      # clique tiles per PSUM group (4 * 128 = 512 = one PSUM bank)
GW = GROUP * P  # 512
SLAB = 4096    # stream slab width (cols); 8 groups per slab

HEIGHTS = (128, 96, 64, 32)

# pool depths (module-level so experiments can override)
BUFS = dict(xs=5, tls=5, ohp=5, outs=4, g128=5, g96=4, g64=4, g32=4)

_COMPILE_CACHE: dict = {}


# --------------------------------------------------------------------------
# planning (shared across cores -> one SPMD program)
# --------------------------------------------------------------------------

def _plan(cnts_list, kmax):
    """Build the uniform class/tile/group/chunk structure from per-core
    per-clique edge counts."""
    K = kmax
    ncls = np.zeros((len(cnts_list), K + 1), np.int64)
    for c, cnt in enumerate(cnts_list):
        b = np.bincount(cnt, minlength=K + 1)
        ncls[c, : len(b)] = b[: K + 1]
    # tiles per class: max over cores, so the program is core-independent
    n = [int(max((ncls[c, k] + P - 1) // P for c in range(len(cnts_list))))
         for k in range(K + 1)]
    n[0] += (-n[0]) % GROUP  # class-0 section group-aligned (bias homogeneity)
    rest = sum(n[1:])
    if rest % GROUP:
        kf = min(k for k in range(1, K + 1) if n[k] > 0)
        n[kf] += (-rest) % GROUP

    classes = [k for k in range(K + 1) if n[k] > 0]  # 0 first, then ascending
    tiles = []           # global tile list -> class k
    class_tile0 = {}     # class -> first global tile index
    for k in classes:
        class_tile0[k] = len(tiles)
        tiles += [k] * n[k]
    n_t = len(tiles)
    assert n_t % GROUP == 0

    # chunk-stream allocation: exact heights (h slots -> h*32 partitions)
    c = {h: 0 for h in HEIGHTS}
    groups = []
    tile_chunks = [[] for _ in range(n_t)]   # tile -> [(H, col128)] (host)
    for gi in range(n_t // GROUP):
        ks = tiles[gi * GROUP:(gi + 1) * GROUP]
        uniform = len(set(ks)) == 1
        grp = dict(off=gi * GW, ks=ks, uniform=uniform,
                   bias0=(ks[0] == 0), mm128=[], mmrem=None)
        if uniform:
            k = ks[0]
            if k > 0:
                full, rem = k // 4, k % 4
                c[128] += (-c[128]) % GW
                grp["mm128"] = [c[128] + j * GW for j in range(full)]
                c[128] += full * GW
                if rem:
                    H = 32 * rem
                    c[H] += (-c[H]) % GW
                    grp["mmrem"] = (H, c[H])
                    c[H] += GW
                for tl in range(GROUP):
                    t = gi * GROUP + tl
                    for j in range(full):
                        tile_chunks[t].append((128, grp["mm128"][j] + tl * P))
                    if rem:
                        tile_chunks[t].append((32 * rem, grp["mmrem"][1] + tl * P))
        else:
            for tl in range(GROUP):
                t = gi * GROUP + tl
                kt = ks[tl]
                for j in range(kt // 4):
                    tile_chunks[t].append((128, c[128]))
                    c[128] += P
                if kt % 4:
                    H = 32 * (kt % 4)
                    tile_chunks[t].append((H, c[H]))
                    c[H] += P
        groups.append(grp)

    return dict(n=n, classes=classes, class_tile0=class_tile0, tiles=tiles,
                n_t=n_t, np_=n_t * P, groups=groups,
                tile_chunks=tile_chunks, gcols=dict(c))


def _core_arrays(plan, x_c, tl_c, deg_c, crow_s, cnt, n_atoms, glpe_q):
    """Per-core input arrays in the permuted, class-grouped layout."""
    NP = plan["np_"]
    cpc = len(cnt)

    starts = np.zeros(cpc + 1, np.int64)
    np.cumsum(cnt, out=starts[1:])
    crow_pad = np.concatenate([crow_s, [n_atoms]]).astype(np.int64)
    n_e = len(crow_s)

    perm = np.full(NP, -1, np.int64)  # position -> original local clique id
    for k in plan["classes"]:
        ids = np.flatnonzero(cnt == k)
        base = plan["class_tile0"][k] * P
        perm[base:base + len(ids)] = ids

    realpos = np.flatnonzero(perm >= 0)
    realids = perm[realpos]

    xp = np.zeros((NP, HID), BF16)
    xp[realpos] = x_c[realids].astype(BF16)
    tlp = np.zeros((NP, PE), FP8)
    tlp[realpos] = np.nan_to_num(tl_c[realids], nan=0.0).astype(FP8)
    oh = np.zeros((100, NP), FP8)
    oh[deg_c[realids].astype(np.int64), realpos] = FP8(1.0)

    G = {h: np.zeros((h, max(plan["gcols"][h], P)), FP8) for h in HEIGHTS}
    tile_chunks = plan["tile_chunks"]
    for k in plan["classes"]:
        if k == 0:
            continue
        nk = plan["n"][k]
        t0 = plan["class_tile0"][k]
        pos = np.arange(t0 * P, (t0 + nk) * P)
        ids = perm[pos]                                   # [nk*128]
        st = np.where(ids >= 0, starts[ids.clip(0)], 0)
        eidx = st[:, None] + np.arange(k)[None, :]        # [nk*128, k]
        valid = (ids >= 0)[:, None] & np.ones((1, k), bool)
        eidx = np.where(valid, eidx, n_e)
        vals = crow_pad[eidx]                             # atom ids (or pad)
        rows = glpe_q[vals]                               # [nk*128, k, 32] fp8
        rows = rows.reshape(nk, P, k, PE)
        for ti in range(nk):
            t = t0 + ti
            for j, (H, col) in enumerate(tile_chunks[t]):
                s0 = j * 4  # chunks are emitted full-first, 4 slots each
                ns = H // 32
                blk = rows[ti, :, s0:s0 + ns, :]          # [128, ns, 32]
                G[H][:, col:col + P] = \
                    blk.transpose(1, 2, 0).reshape(H, P)
    return dict(
        xT=np.ascontiguousarray(xp.T),
        tlT=np.ascontiguousarray(tlp.T),
        ohT=oh,
        g128=G[128], g96=G[96], g64=G[64], g32=G[32],
    ), realpos, realids


# --------------------------------------------------------------------------
# Bass program
# --------------------------------------------------------------------------

def _build_bass(plan, repeat=None, skip=()):
    import concourse.bass as bass
    import concourse.bacc as bacc
    import concourse.mybir as mybir
    import concourse.tile as tile
    from concourse.masks import make_identity

    f32 = mybir.dt.float32
    bf16 = mybir.dt.bfloat16
    fp8 = mybir.dt.float8e3
    NP = plan["np_"]
    groups = plan["groups"]
    n_groups = len(groups)
    n_slabs = (NP + SLAB - 1) // SLAB
    ks_present = [k for k in plan["classes"] if k >= 1]
    gcols = {h: max(plan["gcols"][h], P) for h in HEIGHTS}

    nc = bacc.Bacc(None)
    d_xT = nc.declare_dram_parameter("xT", [P, NP], bf16, isOutput=False)
    d_tlT = nc.declare_dram_parameter("tlT", [PE, NP], fp8, isOutput=False)
    d_oh = nc.declare_dram_parameter("ohT", [100, NP], fp8, isOutput=False)
    d_g = {h: nc.declare_dram_parameter(f"g{h}", [h, gcols[h]], fp8,
                                        isOutput=False) for h in HEIGHTS}
    d_de = nc.declare_dram_parameter("deg_emb", [100, HID], f32, isOutput=False)
    d_w1 = nc.declare_dram_parameter("w1", [HID, HID], f32, isOutput=False)
    d_b1 = nc.declare_dram_parameter("b1", [HID, 1], f32, isOutput=False)
    d_wm = nc.declare_dram_parameter("wm", [HID, HID], f32, isOutput=False)
    d_mb = nc.declare_dram_parameter("mb", [HID, 1], f32, isOutput=False)
    d_tlw = nc.declare_dram_parameter("tlw", [PE, 64], f32, isOutput=False)
    d_tlb = nc.declare_dram_parameter("tlb", [HID, 1], f32, isOutput=False)
    d_lpw = nc.declare_dram_parameter("lpw", [PE, 64], f32, isOutput=False)
    d_lpb = nc.declare_dram_parameter("lpb", [HID, 1], f32, isOutput=False)
    d_out = nc.declare_dram_parameter("outT", [P, NP], bf16, isOutput=True)

    with tile.TileContext(nc) as tc:
        with (
            tc.tile_pool(name="const", bufs=1) as cp,
            tc.tile_pool(name="xs", bufs=BUFS["xs"]) as xpool,
            tc.tile_pool(name="tls", bufs=BUFS["tls"]) as tlpool,
            tc.tile_pool(name="ohp", bufs=BUFS["ohp"]) as ohpool,
            tc.tile_pool(name="outs", bufs=BUFS["outs"]) as opool,
            tc.tile_pool(name="g128", bufs=BUFS["g128"]) as gp128,
            tc.tile_pool(name="g96", bufs=BUFS["g96"]) as gp96,
            tc.tile_pool(name="g64", bufs=BUFS["g64"]) as gp64,
            tc.tile_pool(name="g32", bufs=BUFS["g32"]) as gp32,
            tc.tile_pool(name="psPre", bufs=1, space="PSUM") as psPre,
            tc.tile_pool(name="psF", bufs=7, space="PSUM") as psF,
        ):
            # ---------------- constants / preamble ----------------
            id_sb = cp.tile([P, P], f32, tag="id128")
            make_identity(nc, id_sb[:])

            de_sb = cp.tile([100, HID], f32, tag="de")
            nc.sync.dma_start(out=de_sb[:], in_=d_de[:, :])
            w1_sb = cp.tile([HID, HID], f32, tag="w1")
            nc.sync.dma_start(out=w1_sb[:], in_=d_w1[:, :])
            wm_sb = cp.tile([HID, HID], f32, tag="wm")
            nc.sync.dma_start(out=wm_sb[:], in_=d_wm[:, :])
            tlw_sb = cp.tile([PE, 64], f32, tag="tlw")
            nc.sync.dma_start(out=tlw_sb[:], in_=d_tlw[:, :])
            lpw_sb = cp.tile([PE, 64], f32, tag="lpw")
            nc.sync.dma_start(out=lpw_sb[:], in_=d_lpw[:, :])
            b1c = cp.tile([HID, 1], f32, tag="b1c")
            nc.sync.dma_start(out=b1c[:], in_=d_b1[:, :])
            mbc = cp.tile([HID, 1], f32, tag="mbc")
            nc.sync.dma_start(out=mbc[:], in_=d_mb[:, :])
            tlbc = cp.tile([HID, 1], f32, tag="tlbc")
            nc.sync.dma_start(out=tlbc[:], in_=d_tlb[:, :])
            lpbc = cp.tile([HID, 1], f32, tag="lpbc")
            nc.sync.dma_start(out=lpbc[:], in_=d_lpb[:, :])

            # bf16 copies of streaming weights
            wm_bf = cp.tile([HID, HID], bf16, tag="wm_bf")
            nc.vector.tensor_copy(wm_bf[:], wm_sb[:])
            tlw_bf = cp.tile([PE, 64], bf16, tag="tlw_bf")
            nc.vector.tensor_copy(tlw_bf[:], tlw_sb[:])

            # T2 = relu(deg_emb @ W1 + b1) @ Wm        [100, 128] bf16
            ps_demT = psPre.tile([P, 100], f32, tag="pre")
            nc.tensor.transpose(out=ps_demT[:], in_=de_sb[:],
                                identity=id_sb[:100, :100])
            demT = cp.tile([P, 100], f32, tag="demT")
            nc.vector.tensor_copy(demT[:], ps_demT[:])
            ps_t1t = psPre.tile([P, 100], f32, tag="pre")
            nc.tensor.matmul(ps_t1t[:], lhsT=w1_sb[:], rhs=demT[:],
                             start=True, stop=True)
            t1t = cp.tile([P, 100], f32, tag="t1t")
            nc.scalar.activation(t1t[:], ps_t1t[:],
                                 mybir.ActivationFunctionType.Relu,
                                 bias=b1c[:, :1])
            ps_t2 = psPre.tile([100, P], f32, tag="pre")
            nc.tensor.matmul(ps_t2[:], lhsT=t1t[:], rhs=wm_sb[:],
                             start=True, stop=True)
            t2_bf = cp.tile([100, P], bf16, tag="t2_bf")
            nc.scalar.copy(t2_bf[:], ps_t2[:])

            # per-class stacked pec weights W4[k] = vstack(lpw x4) * (1/k)
            Sid = cp.tile([PE, P], f32, tag="Sid")
            for j in range(4):
                nc.vector.tensor_copy(Sid[:, j * PE:(j + 1) * PE],
                                      id_sb[:PE, :PE])
            w4 = {}
            for k in ks_present:
                ps_w4 = psPre.tile([P, 64], f32, tag="pre")
                nc.tensor.matmul(ps_w4[:], lhsT=Sid[:], rhs=lpw_sb[:],
                                 start=True, stop=True)
                t = cp.tile([P, 64], bf16, tag=f"w4_{k}")
                nc.scalar.activation(t[:], ps_w4[:],
                                     mybir.ActivationFunctionType.Copy,
                                     scale=float(1.0 / k))
                w4[k] = t

            # bias columns (tlb zero-padded rows 0:64, lpb rows 64:128 zero)
            bias0 = cp.tile([HID, 1], f32, tag="bias0")
            nc.vector.tensor_tensor(out=bias0[:], in0=mbc[:], in1=tlbc[:],
                                    op=mybir.AluOpType.add)
            bias1 = cp.tile([HID, 1], f32, tag="bias1")
            nc.vector.tensor_tensor(out=bias1[:], in0=bias0[:], in1=lpbc[:],
                                    op=mybir.AluOpType.add)

            # ---------------- main loop ----------------
            import contextlib
            rep_ctx = (tc.For_i(0, repeat, 1) if repeat
                       else contextlib.nullcontext())
            rep_ctx.__enter__()

            gpools = {128: gp128, 96: gp96, 64: gp64, 32: gp32}
            g_tiles = {h: {} for h in HEIGHTS}

            def ensure_g(H, gs):
                tiles = g_tiles[H]
                if gs not in tiles:
                    w = min(SLAB, gcols[H] - gs * SLAB)
                    t = gpools[H].tile([H, SLAB], fp8, tag=f"g{H}")
                    nc.sync.dma_start(out=t[:, :w],
                                      in_=d_g[H][:, gs * SLAB:gs * SLAB + w])
                    tiles[gs] = t

            def g_view(H, col, width):
                gs = col // SLAB
                ensure_g(H, gs)
                loc = col - gs * SLAB
                return g_tiles[H][gs][:, loc:loc + width]

            # which G slabs does each xs-slab's group range touch?
            g_needed = [set() for _ in range(n_slabs)]
            for g, grp in enumerate(groups):
                si_g = grp["off"] // SLAB
                for col in grp["mm128"]:
                    g_needed[si_g].add((128, col // SLAB))
                if grp["mmrem"] is not None:
                    H, col = grp["mmrem"]
                    g_needed[si_g].add((H, col // SLAB))
                if not grp["uniform"]:
                    for tl in range(GROUP):
                        t = g * GROUP + tl
                        for (H, col) in plan["tile_chunks"][t]:
                            g_needed[si_g].add((H, col // SLAB))

            for si in range(n_slabs):
                w = min(SLAB, NP - si * SLAB)
                xs = xpool.tile([P, SLAB], bf16, tag="xs")
                nc.sync.dma_start(out=xs[:, :w],
                                  in_=d_xT[:, si * SLAB:si * SLAB + w])
                tls = tlpool.tile([PE, SLAB], fp8, tag="tls")
                nc.sync.dma_start(out=tls[:, :w],
                                  in_=d_tlT[:, si * SLAB:si * SLAB + w])
                ohs = ohpool.tile([100, SLAB], fp8, tag="ohp")
                if "oh" not in skip:
                    nc.sync.dma_start(out=ohs[:, :w],
                                      in_=d_oh[:, si * SLAB:si * SLAB + w])
                outs = opool.tile([P, SLAB], bf16, tag="outs")
                if "pec" not in skip:
                    for (H, gs) in sorted(g_needed[si] | (
                            g_needed[si + 1] if si + 1 < n_slabs else set())):
                        ensure_g(H, gs)

                def group_mms(g):
                    """(fin, [(out_ap, lhsT, rhs)]) for one group's bank."""
                    grp = groups[g]
                    loc = grp["off"] - si * SLAB
                    fin = psF.tile([P, GW], f32, tag="fin")
                    mms = [
                        (fin[:], wm_bf[:], xs[:, loc:loc + GW]),
                        (fin[:], t2_bf[:], ohs[:, loc:loc + GW]),
                        (fin[64:128, :], tlw_bf[:], tls[:, loc:loc + GW]),
                    ]
                    if "oh" in skip:
                        del mms[1]
                    if "pec" in skip:
                        pass
                    elif grp["uniform"]:
                        k = grp["ks"][0]
                        for col in grp["mm128"]:
                            mms.append((fin[0:64, :], w4[k][:],
                                        g_view(128, col, GW)))
                        if grp["mmrem"] is not None:
                            H, col = grp["mmrem"]
                            mms.append((fin[0:64, :], w4[k][0:H, :],
                                        g_view(H, col, GW)))
                    else:
                        for tl in range(GROUP):
                            kt = grp["ks"][tl]
                            if kt == 0:
                                continue
                            t = g * GROUP + tl
                            for (H, col) in plan["tile_chunks"][t]:
                                mms.append((
                                    fin[0:64, tl * P:(tl + 1) * P],
                                    w4[kt][0:H, :], g_view(H, col, P)))
                    return fin, mms

                def drain(g, fin):
                    grp = groups[g]
                    loc = grp["off"] - si * SLAB
                    bias_ap = bias0 if grp["bias0"] else bias1
                    if g % 2 == 0:
                        nc.scalar.activation(
                            outs[:, loc:loc + GW], fin[:],
                            mybir.ActivationFunctionType.Identity,
                            bias=bias_ap[:, :1])
                    else:
                        nc.vector.tensor_scalar(
                            out=outs[:, loc:loc + GW], in0=fin[:],
                            scalar1=bias_ap[:, :1], scalar2=None,
                            op0=mybir.AluOpType.add)

                g0 = si * (SLAB // GW)
                g1 = min(g0 + SLAB // GW, n_groups)
                if "compute" in skip:
                    if "pec" not in skip:
                        for g in range(g0, g1):
                            grp = groups[g]
                            for col in grp["mm128"]:
                                g_view(128, col, GW)
                            if grp["mmrem"] is not None:
                                g_view(*grp["mmrem"], GW)
                            if not grp["uniform"]:
                                for tl in range(GROUP):
                                    t = g * GROUP + tl
                                    for (H, col) in plan["tile_chunks"][t]:
                                        g_view(H, col, P)
                    nc.scalar.dma_start(
                        out=d_out[:, si * SLAB:si * SLAB + w],
                        in_=xs[:, :w])
                    continue
                # quads of groups, matmuls interleaved weight-major so the
                # stationary operand is reused across adjacent banks
                for g in range(g0, g1, 4):
                    quad = [group_mms(gg) for gg in range(g, min(g + 4, g1))]
                    nph = max(len(m) for _, m in quad)
                    for i in range(nph):
                        for fin, mms in quad:
                            if i < len(mms):
                                o, lt, rh = mms[i]
                                nc.tensor.matmul(o, lhsT=lt, rhs=rh,
                                                 start=(i == 0),
                                                 stop=(i == len(mms) - 1),
                                                 skip_group_check=True)
                    for gg, (fin, _) in zip(range(g, g + 4), quad):
                        drain(gg, fin)
                    # store the finished half-slab while the next quad runs
                    h0 = (g - g0) * GW
                    h1 = min(h0 + 4 * GW, w)
                    if h1 > h0:
                        nc.scalar.dma_start(
                            out=d_out[:, si * SLAB + h0:si * SLAB + h1],
                            in_=outs[:, h0:h1])

            rep_ctx.__exit__(None, None, None)

    nc.compile()
    return nc


# --------------------------------------------------------------------------
# entry point
# --------------------------------------------------------------------------

def _run_spmd(nc, in_maps, bench=None):
    """Execute the SPMD program via PJRT (axon). Mirrors
    bass2jax.run_bass_via_pjrt but keeps the compiled callable and
    device-resident inputs so `bench` can time repeated executions."""
    import jax
    import numpy as np
    from jax.sharding import Mesh, PartitionSpec
    from jax.experimental.shard_map import shard_map
    from concourse import bass2jax, mybir
    from concourse.bass2jax import _bass_exec_p, partition_id_tensor

    bass2jax.install_neuronx_cc_hook()
    n_cores = len(in_maps)
    partition_name = nc.partition_id_tensor.name if nc.partition_id_tensor else None
    in_names, out_names, out_avals, zero_outs = [], [], [], []
    for alloc in nc.m.functions[0].allocations:
        if not isinstance(alloc, mybir.MemoryLocationSet):
            continue
        name = alloc.memorylocations[0].name
        if alloc.kind == "ExternalInput":
            if name != partition_name:
                in_names.append(name)
        elif alloc.kind == "ExternalOutput":
            out_names.append(name)
            shape = tuple(alloc.tensor_shape)
            dtype = mybir.dt.np(alloc.dtype)
            out_avals.append(jax.core.ShapedArray(shape, dtype))
            zero_outs.append(np.zeros(shape, dtype))
    n_params = len(in_names)
    n_outs = len(out_avals)
    in_names.extend(out_names)
    if partition_name is not None:
        in_names.append(partition_name)

    def _body(*args):
        operands = list(args)
        if partition_name is not None:
            operands.append(partition_id_tensor())
        return tuple(_bass_exec_p.bind(
            *operands, out_avals=tuple(out_avals), in_names=tuple(in_names),
            out_names=tuple(out_names), lowering_input_output_aliases=(),
            sim_require_finite=True, sim_require_nnan=True, nc=nc))

    devices = jax.devices()[:n_cores]
    mesh = Mesh(np.asarray(devices), ("core",))
    in_specs = (PartitionSpec("core"),) * (n_params + n_outs)
    out_specs = (PartitionSpec("core"),) * len(out_names)
    sharded = jax.jit(shard_map(_body, mesh=mesh, in_specs=in_specs,
                                out_specs=out_specs, check_rep=False),
                      keep_unused=True)
    concat_in = [np.concatenate([np.asarray(m[in_names[i]]) for m in in_maps], axis=0)
                 for i in range(n_params)]
    concat_zeros = [np.zeros((n_cores * z.shape[0], *z.shape[1:]), z.dtype)
                    for z in zero_outs]
    sharding = jax.sharding.NamedSharding(mesh, PartitionSpec("core"))
    dev_in = [jax.device_put(a, sharding) for a in concat_in + concat_zeros]
    out_arrs = jax.block_until_ready(sharded(*dev_in))

    if bench is not None:
        import time
        iters = int(bench.get("iters", 10))
        times = []
        for _ in range(iters):
            t0 = time.perf_counter()
            jax.block_until_ready(sharded(*dev_in))
            times.append(time.perf_counter() - t0)
        bench["times"] = times
        bench["min_wall_ns"] = int(min(times) * 1e9)

    return [{name: np.asarray(out_arrs[i]).reshape(n_cores, *out_avals[i].shape)[c]
             for i, name in enumerate(out_names)} for c in range(n_cores)]


def kernel(x_clique, tree_lpe, graph_lpe, tree_degree, row, col,
           deg_emb, deg_lin_w, deg_lin_b, deg_merge_w, deg_merge_b,
           tree_lpe_w, tree_lpe_b, lpe_w, lpe_b, _bench=None):

    x_clique = np.asarray(x_clique, np.float32)
    tree_lpe = np.asarray(tree_lpe, np.float32)
    graph_lpe = np.asarray(graph_lpe, np.float32)
    tree_degree = np.asarray(tree_degree).astype(np.int64)
    row = np.asarray(row).astype(np.int64)
    col = np.asarray(col).astype(np.int64)

    n_clique = x_clique.shape[0]
    n_atoms = graph_lpe.shape[0]
    assert n_clique % N_CORES == 0
    cpc = n_clique // N_CORES

    # ---- host index prep: partition edges by owning core, count per clique
    order = np.argsort(col, kind="stable")
    col_s = col[order]
    row_s = row[order]
    bounds = np.searchsorted(col_s, np.arange(N_CORES + 1) * cpc)

    cnts, crows = [], []
    for c in range(N_CORES):
        lo, hi = bounds[c], bounds[c + 1]
        cc = col_s[lo:hi] - c * cpc
        cnts.append(np.bincount(cc, minlength=cpc).astype(np.int64))
        crows.append(row_s[lo:hi])

    kmax = int(max(int(c.max(initial=0)) for c in cnts))
    plan = _plan(cnts, kmax)

    glpe_q = np.vstack([np.nan_to_num(graph_lpe, nan=0.0),
                        np.zeros((1, PE), np.float32)]).astype(FP8)

    weights = dict(
        deg_emb=np.ascontiguousarray(deg_emb, np.float32),
        w1=np.ascontiguousarray(deg_lin_w, np.float32),
        b1=np.ascontiguousarray(deg_lin_b.reshape(HID, 1), np.float32),
        wm=np.ascontiguousarray(deg_merge_w, np.float32),
        mb=np.ascontiguousarray(deg_merge_b.reshape(HID, 1), np.float32),
        tlw=np.ascontiguousarray(tree_lpe_w, np.float32),
        tlb=np.concatenate([np.zeros(64, np.float32),
                            np.asarray(tree_lpe_b, np.float32)]).reshape(HID, 1),
        lpw=np.ascontiguousarray(lpe_w, np.float32),
        lpb=np.concatenate([np.asarray(lpe_b, np.float32),
                            np.zeros(64, np.float32)]).reshape(HID, 1),
    )

    in_maps = []
    unshard = []
    for c in range(N_CORES):
        arrs, realpos, realids = _core_arrays(
            plan, x_clique[c * cpc:(c + 1) * cpc],
            tree_lpe[c * cpc:(c + 1) * cpc],
            tree_degree[c * cpc:(c + 1) * cpc],
            crows[c], cnts[c], n_atoms, glpe_q)
        m = dict(**arrs, **weights)
        in_maps.append(m)
        unshard.append((realpos, realids))

    cache_key = (plan["n_t"], tuple(sorted(plan["gcols"].items())),
                 tuple(plan["tiles"]))
    nc = _COMPILE_CACHE.get(cache_key)
    if nc is None:
        nc = _build_bass(plan)
        _COMPILE_CACHE[cache_key] = nc

    results = _run_spmd(nc, in_maps, bench=_bench)

    # true HW time: run repeat-R variants of the program (device-side loop);
    # the wall-time slope vs R is pure device time, dispatch cancels out.
    if _bench is not None and _bench.get("hw_probe"):
        walls = {}
        for R in _bench["hw_probe"]:
            ncR = _build_bass(plan, repeat=R)
            b2 = {"iters": _bench.get("iters", 8)}
            _run_spmd(ncR, in_maps, bench=b2)
            walls[R] = min(b2["times"])
        rs = sorted(walls)
        _bench["walls"] = walls
        _bench["hw_ns_est"] = int(
            (walls[rs[-1]] - walls[rs[0]]) / (rs[-1] - rs[0]) * 1e9)

    out = np.empty((n_clique, HID), np.float32)
    for c in range(N_CORES):
        realpos, realids = unshard[c]
        outT = results[c]["outT"]  # [128, NP] bf16
        out[c * cpc + realids] = outT.T[realpos].astype(np.float32)
    return out
